# revision 21
# baseline (speedup 1.0000x reference)
"""HeadConvAttention Trainium2 Bass kernel (transposed-orientation design).

Reference computation (per batch b):
    scores[h,q,k] = <xq[h,q,:], xk[h,k,:]> / sqrt(D)
    mixed[g,q,k]  = sum_h W[g,h] * scores[h,q,k]
    probs         = softmax(mixed + causal_mask, axis=k)
    out[q,g,d]    = sum_k probs[g,q,k] * xv[g,k,d]

Sharding: 8 cores = batch(4) x q-parity(2). Each core handles q rows
{parity, parity+2, ...} (512 rows) of one batch element — strided q keeps
the causal workload balanced across parities.

Everything is kept in the [k, q] ("transposed") orientation so probs come
out of the mix+exp already in the layout PV consumes — no post-softmax
transposes and no PSUM->SBUF probs copies:

  - host pre-transposes inputs: qt [pair, (2h x d), qlocal] bf16,
    kt [pair, (2h x d), k] bf16, vt [kp, g, kblk, d+ones] bf16; W enters as
    the 128x128 block-diag wblk (x 1/8 scale), causal diag mask cm2 and the
    128x128 identity also come from the host.
  - QK^T per (kblk, head pair): lhsT = kt slice [64, 128k], rhs = qt
    [64, q>=64*kblk] -> scoresT [k=128, q] f32 in PSUM, two row-tiled
    matmuls per pair (tile_position (0,0)/(64,0)).
  - scatter (DVE): PSUM -> sc [k, qb, h, qc] bf16 (qb = q//8, qc = q%8).
  - fold (PE transpose per 8-q block): [k, (h qc)] -> [(h qc), k] bf16,
    batched 8 per PSUM bank, one DVE copy out.
  - mix matmul: lhsT = fold slice, rhs = block-diag W -> pm [k, (g qc)]
    f32; exp on ScalarE -> probsT[kblk] [k, g, qb, qc] bf16 (PV-ready,
    (qb, qc) contiguous per g so the PV rhs is a single free dim).
  - causal diagonal masked multiplicatively (mask indep. of kblk:
    kp <= 16*qbr + 2*qc + par); kblk<6 on otherwise-idle GpSimd, the last
    two kblks on DVE so PV is not gated on the slow gpsimd chain.
  - PV per g: lhsT = vt[:, g, kblk, :] (V with a ones column, stationary),
    rhs = probsT[kblk][:, g] streamed (N up to 512), PSUM-accumulated over
    kblk into po [65, q] — row 64 is the softmax denominator. po banks
    alternate between the ps_qk and ps_mix pools (both idle by then) so
    the drain ring never stalls the PE.
  - po -> SBUF -> DMA out [g, 65, q]; host does out = po[:64]/po[64] and
    the final [B, S, H, D] gather.

Emission is software-pipelined (engine queues run in program order):
QKT(kblk+1) is emitted before folds(kblk) so the PE queue has work while
DVE drains the QKT PSUM banks; fold groups and mix groups interleave.
Matmul-weights APs must have a single free dim, and PSUM tiles are
bank-sized — these constraints fix the sc/probsT layouts above.
"""

import numpy as np

B, H, S, D = 4, 16, 1024, 64
QC = S // 2          # q rows per core
NP = 8               # head pairs

_compiled = {}
TRACE = False          # set True to capture an NTFF profile on the next call
LAST_EXEC_NS = None
LAST_PROFILE = None


def _build_nc(causal: int):
    import contextlib

    import concourse.bacc as bacc
    import concourse.bass as bass
    import concourse.mybir as mybir
    import concourse.tile as tile

    dt = mybir.dt
    f32, bf16 = dt.float32, dt.bfloat16
    AF = mybir.ActivationFunctionType

    nc = bacc.Bacc("TRN2", target_bir_lowering=False, debug=False, num_devices=8)

    qt_c = nc.dram_tensor("qt_c", [128, NP, QC], bf16, kind="ExternalInput")
    kt_c = nc.dram_tensor("kt_c", [128, 8, NP, 128], bf16, kind="ExternalInput")
    vt_c = nc.dram_tensor("vt_c", [128, H, 8, 65], bf16, kind="ExternalInput")
    wblk = nc.dram_tensor("wblk", [128, 128], bf16, kind="ExternalInput")
    cmask = nc.dram_tensor("cmask", [128, 8, 128], bf16, kind="ExternalInput")
    ident = nc.dram_tensor("ident", [128, 128], bf16, kind="ExternalInput")
    out_c = nc.dram_tensor("out_c", [H, 65, QC], f32, kind="ExternalOutput")

    # q windows (qb units of 8 local q rows). causal=1 fits one window in
    # SBUF because probsT shrinks with kblk; causal=0 needs two passes.
    if causal:
        windows = [(0, QC // 8)]
    else:
        windows = [(0, QC // 16), (QC // 16, QC // 8)]

    with tile.TileContext(nc) as tc:
        with contextlib.ExitStack() as ctx:
            const = ctx.enter_context(tc.tile_pool(name="const", bufs=1))
            persist = ctx.enter_context(tc.tile_pool(name="persist", bufs=1))
            sc_pool = ctx.enter_context(tc.tile_pool(name="scores", bufs=2))
            fold_pool = ctx.enter_context(tc.tile_pool(name="fold", bufs=3))
            ob_pool = ctx.enter_context(tc.tile_pool(name="outp", bufs=3))
            # PSUM budget: 8 banks. qk/po 3 + ft 2 + mix 3 = 8.
            ps_qk = ctx.enter_context(tc.tile_pool(name="ps_qk", bufs=3, space="PSUM"))
            ps_ft = ctx.enter_context(tc.tile_pool(name="ps_ft", bufs=2, space="PSUM"))
            ps_mix = ctx.enter_context(tc.tile_pool(name="ps_mix", bufs=3, space="PSUM"))

            # ---- input loads (batched; kblk-0 K slice + first qt half first) ----
            qt_a = persist.tile([128, 4, QC], bf16, tag="qt_a")
            qt_b = persist.tile([128, 4, QC], bf16, tag="qt_b")
            kt0_sb = persist.tile([128, NP, 128], bf16, tag="kt0")
            ktr_sb = persist.tile([128, 7, NP, 128], bf16, tag="ktr")
            # identity first (32KB, negligible delay) so PE warmup can
            # bring the HAM clock to 2.4 GHz while the real inputs load
            id_bf = const.tile([128, 128], bf16, tag="id_bf")
            nc.sync.dma_start(out=id_bf, in_=ident[:, :])
            nc.sync.dma_start(out=kt0_sb, in_=kt_c[:, 0, :, :])
            nc.sync.dma_start(out=qt_a, in_=qt_c[:, 0:4, :])
            nc.sync.dma_start(out=qt_b, in_=qt_c[:, 4:8, :])
            warm = ps_mix.tile([128, 512], f32, tag="pm", name="warm")
            for _ in range(40):
                nc.tensor.matmul(
                    warm[:, 0:128], id_bf, id_bf, start=True, stop=True
                )

            # ---- constants ----
            wblk_bf = const.tile([128, 128], bf16, tag="wblk_bf")
            nc.sync.dma_start(out=wblk_bf, in_=wblk[:, :])
            cm2 = const.tile([128, 8, 128], bf16, tag="cm2")

            def qt_slice(hl, p, c0, c1):
                t = qt_a if p < 4 else qt_b
                return t[64 * hl : 64 * (hl + 1), p % 4, c0:c1]
            nc.sync.dma_start(
                out=ktr_sb.rearrange("k a p j -> k (a p j)"),
                in_=kt_c[:, 1:8, :, :].rearrange("k a p j -> k (a p j)"),
            )
            nc.sync.dma_start(out=cm2, in_=cmask[:, :, :])
            vt_sb = persist.tile([128, H, 8, 65], bf16, tag="vt")
            nc.sync.dma_start(out=vt_sb, in_=vt_c[:, :, :, :])

            def kt_slice(p, kblk):
                if kblk == 0:
                    return kt0_sb[:, p, :]
                return ktr_sb[:, kblk - 1, p, :]

            def run_window(widx, qbw0, qbw1):
                # probsT[kblk]: [k, g, qbr, qc] bf16, qbr local to window
                probsT = {}
                sc_tiles = {}
                q_emitted = set()

                def qb0_of(kblk):
                    return max(qbw0, 8 * kblk) if causal else qbw0

                def emit_qkt_pair(kblk, p):
                    qb0 = qb0_of(kblk)
                    nqb = qbw1 - qb0
                    nq = 8 * nqb
                    if kblk not in sc_tiles:
                        sc_tiles[kblk] = sc_pool.tile(
                            [128, qbw1 - qbw0, H, 8], bf16, tag="sc", name="sc"
                        )
                    sc = sc_tiles[kblk]
                    pq0 = ps_qk.tile([128, 512], f32, tag="pq", name="pq0")[:, 0:nq]
                    pq1 = ps_qk.tile([128, 512], f32, tag="pq", name="pq1")[:, 0:nq]
                    nc.tensor.matmul(
                        pq0,
                        kt_slice(p, kblk)[0:64, :],
                        qt_slice(0, p, 8 * qb0, 8 * qbw1),
                        start=True,
                        stop=True,
                        tile_position=(0, 0),
                    )
                    nc.tensor.matmul(
                        pq1,
                        kt_slice(p, kblk)[64:128, :],
                        qt_slice(1, p, 8 * qb0, 8 * qbw1),
                        start=True,
                        stop=True,
                        tile_position=(64, 0),
                    )
                    src0 = pq0.rearrange("k (qb qc) -> k qb qc", qc=8)
                    src1 = pq1.rearrange("k (qb qc) -> k qb qc", qc=8)
                    nc.vector.tensor_copy(sc[:, 0:nqb, 2 * p, :], src0)
                    nc.vector.tensor_copy(sc[:, 0:nqb, 2 * p + 1, :], src1)

                def ensure_q(kblk):
                    if 0 <= kblk < 8 and kblk not in q_emitted:
                        for p in range(NP):
                            emit_qkt_pair(kblk, p)
                        q_emitted.add(kblk)

                def emit_fold_grp(kblk, grp):
                    sc = sc_tiles[kblk]
                    ft = ps_ft.tile([128, 8, 128], bf16, tag="ft", name="ft")
                    for i in range(8):
                        nc.tensor.transpose(
                            ft[:, i, :],
                            sc[:, grp * 8 + i, :, :].rearrange("k h qc -> k (h qc)"),
                            id_bf,
                        )
                    fold_sb = fold_pool.tile([128, 8, 128], bf16, tag="fold", name="fold_sb")
                    # middle kblks are ScalarE-bound (exp) -> lean on DVE there
                    s = 6 if 3 <= kblk <= 6 else 4
                    nc.vector.tensor_copy(fold_sb[:, 0:s, :], ft[:, 0:s, :])
                    nc.scalar.copy(fold_sb[:, s:8, :], ft[:, s:8, :])
                    return fold_sb

                def emit_mix_grp(kblk, grp, fold_sb):
                    pt = probsT[kblk]
                    for half in range(2):
                        pm = ps_mix.tile([128, 512], f32, tag="pm", name="pm")
                        for i in range(4):
                            nc.tensor.matmul(
                                pm[:, 128 * i : 128 * (i + 1)],
                                fold_sb[:, half * 4 + i, :],
                                wblk_bf,
                                start=True,
                                stop=True,
                            )
                        qbr0 = grp * 8 + half * 4
                        nc.scalar.activation(
                            pt[:, qbr0 : qbr0 + 4, :, :].rearrange(
                                "k qb g qc -> k qb (g qc)"
                            ),
                            pm.rearrange("k (qb gq) -> k qb gq", gq=128),
                            AF.Exp,
                        )

                def emit_scores(kblk):
                    qb0 = qb0_of(kblk)
                    nqb = qbw1 - qb0
                    probsT[kblk] = persist.tile(
                        [128, nqb, H, 8], bf16, tag=f"pt_{widx}_{kblk}",
                        name=f"pt_{widx}_{kblk}",
                    )
                    ngrp = nqb // 8
                    # interleave next kblk's QKT pairs between fold groups so
                    # the PE queue never blocks on the pq ring at boundaries
                    nxt = kblk + 1
                    do_next = nxt < 8 and nxt not in q_emitted
                    pend = None
                    for grp in range(ngrp):
                        fold_sb = emit_fold_grp(kblk, grp)
                        if do_next:
                            lo = NP * grp // ngrp
                            hi = NP * (grp + 1) // ngrp
                            for p in range(lo, hi):
                                emit_qkt_pair(nxt, p)
                        if pend is not None:
                            emit_mix_grp(kblk, pend[0], pend[1])
                        pend = (grp, fold_sb)
                    if do_next:
                        q_emitted.add(nxt)
                    if pend is not None:
                        emit_mix_grp(kblk, pend[0], pend[1])
                    sc_tiles.pop(kblk)
                    if causal and 8 * kblk >= qbw0:
                        # diagonal 8 qb straddle the causal boundary; the two
                        # last kblks go to DVE so PV is not gated on the slow
                        # gpsimd mask chain
                        pt = probsT[kblk]
                        if kblk >= 6:
                            cmb = cm2[:, :, :].rearrange(
                                "k qb (g qc) -> k qb g qc", qc=8
                            )
                            nc.vector.tensor_mul(
                                pt[:, 0:8, :, :], pt[:, 0:8, :, :], cmb
                            )
                        else:
                            for qbr in range(8):
                                sl = pt[:, qbr, :, :].rearrange(
                                    "k g qc -> k (g qc)"
                                )
                                nc.gpsimd.tensor_mul(sl, sl, cm2[:, qbr, :])

                for kblk in range(8):
                    ensure_q(kblk)
                    emit_scores(kblk)

                # ---- PV ----
                nw = 8 * (qbw1 - qbw0)
                ob = None
                for g in range(H):
                    pool = ps_qk if g % 2 == 0 else ps_mix
                    tg = "pq" if g % 2 == 0 else "pm"
                    po = pool.tile([128, 512], f32, tag=tg, name="po")
                    for kblk in range(8):
                        qb0 = qb0_of(kblk)
                        nc.tensor.matmul(
                            po[0:65, 8 * (qb0 - qbw0) : 8 * (qbw1 - qbw0)],
                            vt_sb[:, g, kblk, :],
                            probsT[kblk][:, :, g, :],
                            start=(kblk == 0),
                            stop=(kblk == 7),
                        )
                    if g % 4 == 0:
                        ob = ob_pool.tile([128, 4, 512], f32, tag="ob", name="ob")
                    hw_ = nw // 2
                    nc.vector.tensor_copy(ob[0:65, g % 4, 0:hw_], po[0:65, 0:hw_])
                    nc.scalar.copy(ob[0:65, g % 4, hw_:nw], po[0:65, hw_:nw])
                    if g == 13:
                        nc.sync.dma_start(
                            out=out_c[12:14, :, 8 * qbw0 : 8 * qbw1].rearrange(
                                "gg d q -> d gg q"
                            ),
                            in_=ob[0:65, 0:2, 0:nw],
                        )
                    elif g == 15:
                        nc.sync.dma_start(
                            out=out_c[14:16, :, 8 * qbw0 : 8 * qbw1].rearrange(
                                "gg d q -> d gg q"
                            ),
                            in_=ob[0:65, 2:4, 0:nw],
                        )
                    elif g % 4 == 3:
                        nc.sync.dma_start(
                            out=out_c[g - 3 : g + 1, :, 8 * qbw0 : 8 * qbw1].rearrange(
                                "gg d q -> d gg q"
                            ),
                            in_=ob[0:65, :, 0:nw],
                        )

            for widx, (qbw0, qbw1) in enumerate(windows):
                run_window(widx, qbw0, qbw1)

    nc.compile()
    return nc


def _get_nc(causal: int):
    key = int(causal)
    if key not in _compiled:
        _compiled[key] = _build_nc(key)
    return _compiled[key]


def kernel(xq, xk, xv, W, causal):
    import ml_dtypes
    from concourse.bass_utils import run_bass_kernel_spmd

    bf16 = ml_dtypes.bfloat16
    causal = int(np.asarray(causal))
    nc = _get_nc(causal)

    W = np.asarray(W, dtype=np.float32)
    # block-diagonal mixing weight: wblk[8h+qc, 8g+qc] = W[g,h] / 8
    wblk = np.zeros((128, 128), dtype=np.float32)
    for qc in range(8):
        wblk[qc::8, qc::8] = W.T / 8.0
    wblk = wblk.astype(bf16)
    ident = np.eye(128, dtype=np.float32).astype(bf16)

    xq = np.asarray(xq, dtype=np.float32)
    xk = np.asarray(xk, dtype=np.float32)
    xv = np.asarray(xv, dtype=np.float32)

    in_maps = []
    for cid in range(8):
        b, par = divmod(cid, 2)
        # qt[hl*64+d, p, ql] = xq[b, 2p+hl, 2*ql+par, d]
        qt = np.ascontiguousarray(
            xq[b, :, par::2, :]
            .reshape(NP, 2, QC, 64)
            .transpose(1, 3, 0, 2)
            .reshape(128, NP, QC),
            dtype=bf16,
        )
        # kt[hl*64+d, kblk, p, j] = xk[b, 2p+hl, 128*kblk+j, d]
        kt = np.ascontiguousarray(
            xk[b]
            .reshape(NP, 2, 8, 128, 64)
            .transpose(1, 4, 2, 0, 3)
            .reshape(128, 8, NP, 128),
            dtype=bf16,
        )
        # vt[kp, g, kblk, 0:64] = xv[b, g, 128*kblk+kp, :]; vt[..., 64] = 1
        v = xv[b].reshape(H, 8, 128, D).transpose(2, 0, 1, 3)
        vt = np.concatenate(
            [v, np.ones((128, H, 8, 1), dtype=np.float32)], axis=3
        ).astype(bf16)
        # diag mask (g-replicated): cm[kp, qbr, g*8+qc] = 1 if
        # kp <= 16*qbr + 2*qc + par
        kp = np.arange(128)[:, None, None]
        qbr = np.arange(8)[None, :, None]
        qcc = np.arange(8)[None, None, :]
        cm8 = np.where(kp <= 16 * qbr + 2 * qcc + par, 1.0, 0.0).astype(np.float32)
        cm = np.ascontiguousarray(
            np.tile(cm8[:, :, None, :], (1, 1, H, 1)).reshape(128, 8, 128)
        ).astype(bf16)
        in_maps.append(
            {
                "qt_c": qt,
                "kt_c": kt,
                "vt_c": vt,
                "wblk": wblk,
                "cmask": cm,
                "ident": ident,
            }
        )

    global LAST_EXEC_NS, LAST_PROFILE
    res = run_bass_kernel_spmd(nc, in_maps, list(range(8)), trace=TRACE)
    if res.exec_time_ns is not None:
        LAST_EXEC_NS = res.exec_time_ns
        LAST_PROFILE = res.profile_json
    out = np.empty((B, S, H, D), dtype=np.float32)
    for cid in range(8):
        b, par = divmod(cid, 2)
        oc = res.results[cid]["out_c"]  # [H, 65, QC] f32
        o = oc[:, 0:64, :] / oc[:, 64:65, :]
        out[b, par::2, :, :] = o.transpose(2, 0, 1)
    return out


# revision 22
# speedup vs baseline: 1.0774x; 1.0774x over previous
"""HeadConvAttention Trainium2 Bass kernel (transposed-orientation design).

Reference computation (per batch b):
    scores[h,q,k] = <xq[h,q,:], xk[h,k,:]> / sqrt(D)
    mixed[g,q,k]  = sum_h W[g,h] * scores[h,q,k]
    probs         = softmax(mixed + causal_mask, axis=k)
    out[q,g,d]    = sum_k probs[g,q,k] * xv[g,k,d]

Sharding: 8 cores = batch(4) x q-parity(2). Each core handles q rows
{parity, parity+2, ...} (512 rows) of one batch element — strided q keeps
the causal workload balanced across parities.

Everything is kept in the [k, q] ("transposed") orientation so probs come
out of the mix+exp already in the layout PV consumes — no post-softmax
transposes and no PSUM->SBUF probs copies:

  - host pre-transposes inputs: qt [pair, (2h x d), qlocal] bf16,
    kt [pair, (2h x d), k] bf16, vt [kp, g, kblk, d+ones] bf16; W enters as
    the 128x128 block-diag wblk (x 1/8 scale), causal diag mask cm2 and the
    128x128 identity also come from the host.
  - QK^T per (kblk, head pair): lhsT = kt slice [64, 128k], rhs = qt
    [64, q>=64*kblk] -> scoresT [k=128, q] f32 in PSUM, two row-tiled
    matmuls per pair (tile_position (0,0)/(64,0)).
  - scatter (DVE): PSUM -> sc [k, qb, h, qc] bf16 (qb = q//8, qc = q%8).
  - fold (PE transpose per 8-q block): [k, (h qc)] -> [(h qc), k] bf16,
    batched 8 per PSUM bank, one DVE copy out.
  - mix matmul: lhsT = fold slice, rhs = block-diag W -> pm [k, (g qc)]
    f32; exp on ScalarE -> probsT[kblk] [k, g, qb, qc] bf16 (PV-ready,
    (qb, qc) contiguous per g so the PV rhs is a single free dim).
  - causal diagonal masked multiplicatively (mask indep. of kblk:
    kp <= 16*qbr + 2*qc + par); kblk<6 on otherwise-idle GpSimd, the last
    two kblks on DVE so PV is not gated on the slow gpsimd chain.
  - PV per g: lhsT = vt[:, g, kblk, :] (V with a ones column, stationary),
    rhs = probsT[kblk][:, g] streamed (N up to 512), PSUM-accumulated over
    kblk into po [65, q] — row 64 is the softmax denominator. po banks
    alternate between the ps_qk and ps_mix pools (both idle by then) so
    the drain ring never stalls the PE.
  - po -> SBUF -> DMA out [g, 65, q]; host does out = po[:64]/po[64] and
    the final [B, S, H, D] gather.

Emission is software-pipelined (engine queues run in program order):
QKT(kblk+1) is emitted before folds(kblk) so the PE queue has work while
DVE drains the QKT PSUM banks; fold groups and mix groups interleave.
Matmul-weights APs must have a single free dim, and PSUM tiles are
bank-sized — these constraints fix the sc/probsT layouts above.
"""

import numpy as np

B, H, S, D = 4, 16, 1024, 64
QC = S // 2          # q rows per core
NP = 8               # head pairs

_compiled = {}
TRACE = False          # set True to capture an NTFF profile on the next call
LAST_EXEC_NS = None
LAST_PROFILE = None


def _build_nc(causal: int):
    import contextlib

    import concourse.bacc as bacc
    import concourse.bass as bass
    import concourse.mybir as mybir
    import concourse.tile as tile

    dt = mybir.dt
    f32, bf16 = dt.float32, dt.bfloat16
    AF = mybir.ActivationFunctionType

    nc = bacc.Bacc("TRN2", target_bir_lowering=False, debug=False, num_devices=8)

    qt_c = nc.dram_tensor("qt_c", [128, NP, QC], bf16, kind="ExternalInput")
    kt_c = nc.dram_tensor("kt_c", [128, 8, NP, 128], bf16, kind="ExternalInput")
    vt_c = nc.dram_tensor("vt_c", [128, H, 8, 65], bf16, kind="ExternalInput")
    wblk = nc.dram_tensor("wblk", [128, 128], bf16, kind="ExternalInput")
    cmask = nc.dram_tensor("cmask", [128, 8, 128], bf16, kind="ExternalInput")
    ident = nc.dram_tensor("ident", [128, 128], bf16, kind="ExternalInput")
    out_c = nc.dram_tensor("out_c", [H, 65, QC], f32, kind="ExternalOutput")

    # q windows (qb units of 8 local q rows). causal=1 fits one window in
    # SBUF because probsT shrinks with kblk; causal=0 needs two passes.
    if causal:
        windows = [(0, QC // 8)]
    else:
        windows = [(0, QC // 16), (QC // 16, QC // 8)]

    with tile.TileContext(nc) as tc:
        with contextlib.ExitStack() as ctx:
            const = ctx.enter_context(tc.tile_pool(name="const", bufs=1))
            persist = ctx.enter_context(tc.tile_pool(name="persist", bufs=1))
            sc_pool = ctx.enter_context(tc.tile_pool(name="scores", bufs=2))
            fold_pool = ctx.enter_context(tc.tile_pool(name="fold", bufs=3))
            ob_pool = ctx.enter_context(tc.tile_pool(name="outp", bufs=3))
            # PSUM budget: 8 banks. qk/po 3 + ft 2 + mix 3 = 8.
            ps_qk = ctx.enter_context(tc.tile_pool(name="ps_qk", bufs=3, space="PSUM"))
            ps_ft = ctx.enter_context(tc.tile_pool(name="ps_ft", bufs=2, space="PSUM"))
            ps_mix = ctx.enter_context(tc.tile_pool(name="ps_mix", bufs=3, space="PSUM"))

            # ---- input loads (batched; kblk-0 K slice + first qt half first) ----
            qt_a = persist.tile([128, 4, QC], bf16, tag="qt_a")
            qt_b = persist.tile([128, 4, QC], bf16, tag="qt_b")
            kt0_sb = persist.tile([128, NP, 128], bf16, tag="kt0")
            ktr_sb = persist.tile([128, 7, NP, 128], bf16, tag="ktr")
            nc.sync.dma_start(out=kt0_sb, in_=kt_c[:, 0, :, :])
            nc.sync.dma_start(out=qt_a, in_=qt_c[:, 0:4, :])
            nc.sync.dma_start(out=qt_b, in_=qt_c[:, 4:8, :])

            # ---- constants ----
            id_bf = const.tile([128, 128], bf16, tag="id_bf")
            nc.sync.dma_start(out=id_bf, in_=ident[:, :])
            wblk_bf = const.tile([128, 128], bf16, tag="wblk_bf")
            nc.sync.dma_start(out=wblk_bf, in_=wblk[:, :])
            cm2 = const.tile([128, 8, 128], bf16, tag="cm2")

            def qt_slice(hl, p, c0, c1):
                t = qt_a if p < 4 else qt_b
                return t[64 * hl : 64 * (hl + 1), p % 4, c0:c1]
            nc.sync.dma_start(
                out=ktr_sb.rearrange("k a p j -> k (a p j)"),
                in_=kt_c[:, 1:8, :, :].rearrange("k a p j -> k (a p j)"),
            )
            nc.sync.dma_start(out=cm2, in_=cmask[:, :, :])
            vt_sb = persist.tile([128, H, 8, 65], bf16, tag="vt")
            nc.sync.dma_start(out=vt_sb, in_=vt_c[:, :, :, :])

            def kt_slice(p, kblk):
                if kblk == 0:
                    return kt0_sb[:, p, :]
                return ktr_sb[:, kblk - 1, p, :]

            def run_window(widx, qbw0, qbw1):
                # probsT[kblk]: [k, g, qbr, qc] bf16, qbr local to window
                probsT = {}
                sc_tiles = {}
                q_emitted = set()

                def qb0_of(kblk):
                    return max(qbw0, 8 * kblk) if causal else qbw0

                def emit_qkt_pair(kblk, p):
                    qb0 = qb0_of(kblk)
                    nqb = qbw1 - qb0
                    nq = 8 * nqb
                    if kblk not in sc_tiles:
                        sc_tiles[kblk] = sc_pool.tile(
                            [128, qbw1 - qbw0, H, 8], bf16, tag="sc", name="sc"
                        )
                    sc = sc_tiles[kblk]
                    pq0 = ps_qk.tile([128, 512], f32, tag="pq", name="pq0")[:, 0:nq]
                    pq1 = ps_qk.tile([128, 512], f32, tag="pq", name="pq1")[:, 0:nq]
                    nc.tensor.matmul(
                        pq0,
                        kt_slice(p, kblk)[0:64, :],
                        qt_slice(0, p, 8 * qb0, 8 * qbw1),
                        start=True,
                        stop=True,
                        tile_position=(0, 0),
                    )
                    nc.tensor.matmul(
                        pq1,
                        kt_slice(p, kblk)[64:128, :],
                        qt_slice(1, p, 8 * qb0, 8 * qbw1),
                        start=True,
                        stop=True,
                        tile_position=(64, 0),
                    )
                    src0 = pq0.rearrange("k (qb qc) -> k qb qc", qc=8)
                    src1 = pq1.rearrange("k (qb qc) -> k qb qc", qc=8)
                    nc.vector.tensor_copy(sc[:, 0:nqb, 2 * p, :], src0)
                    nc.vector.tensor_copy(sc[:, 0:nqb, 2 * p + 1, :], src1)

                def ensure_q(kblk):
                    if 0 <= kblk < 8 and kblk not in q_emitted:
                        for p in range(NP):
                            emit_qkt_pair(kblk, p)
                        q_emitted.add(kblk)

                def emit_fold_grp(kblk, grp):
                    sc = sc_tiles[kblk]
                    ft = ps_ft.tile([128, 8, 128], bf16, tag="ft", name="ft")
                    for i in range(8):
                        nc.tensor.transpose(
                            ft[:, i, :],
                            sc[:, grp * 8 + i, :, :].rearrange("k h qc -> k (h qc)"),
                            id_bf,
                        )
                    fold_sb = fold_pool.tile([128, 8, 128], bf16, tag="fold", name="fold_sb")
                    # middle kblks are ScalarE-bound (exp) -> lean on DVE there
                    s = 6 if 3 <= kblk <= 6 else 4
                    nc.vector.tensor_copy(fold_sb[:, 0:s, :], ft[:, 0:s, :])
                    nc.scalar.copy(fold_sb[:, s:8, :], ft[:, s:8, :])
                    return fold_sb

                def emit_mix_grp(kblk, grp, fold_sb):
                    pt = probsT[kblk]
                    for half in range(2):
                        pm = ps_mix.tile([128, 512], f32, tag="pm", name="pm")
                        for i in range(4):
                            nc.tensor.matmul(
                                pm[:, 128 * i : 128 * (i + 1)],
                                fold_sb[:, half * 4 + i, :],
                                wblk_bf,
                                start=True,
                                stop=True,
                            )
                        qbr0 = grp * 8 + half * 4
                        nc.scalar.activation(
                            pt[:, qbr0 : qbr0 + 4, :, :].rearrange(
                                "k qb g qc -> k qb (g qc)"
                            ),
                            pm.rearrange("k (qb gq) -> k qb gq", gq=128),
                            AF.Exp,
                        )

                def emit_scores(kblk):
                    qb0 = qb0_of(kblk)
                    nqb = qbw1 - qb0
                    probsT[kblk] = persist.tile(
                        [128, nqb, H, 8], bf16, tag=f"pt_{widx}_{kblk}",
                        name=f"pt_{widx}_{kblk}",
                    )
                    ngrp = nqb // 8
                    # interleave next kblk's QKT pairs between fold groups so
                    # the PE queue never blocks on the pq ring at boundaries
                    nxt = kblk + 1
                    do_next = nxt < 8 and nxt not in q_emitted
                    pend = None
                    for grp in range(ngrp):
                        fold_sb = emit_fold_grp(kblk, grp)
                        if do_next:
                            lo = NP * grp // ngrp
                            hi = NP * (grp + 1) // ngrp
                            for p in range(lo, hi):
                                emit_qkt_pair(nxt, p)
                        if pend is not None:
                            emit_mix_grp(kblk, pend[0], pend[1])
                        pend = (grp, fold_sb)
                    if do_next:
                        q_emitted.add(nxt)
                    if pend is not None:
                        emit_mix_grp(kblk, pend[0], pend[1])
                    sc_tiles.pop(kblk)
                    if causal and 8 * kblk >= qbw0:
                        # diagonal 8 qb straddle the causal boundary; the two
                        # last kblks go to DVE so PV is not gated on the slow
                        # gpsimd mask chain
                        pt = probsT[kblk]
                        if kblk >= 6:
                            cmb = cm2[:, :, :].rearrange(
                                "k qb (g qc) -> k qb g qc", qc=8
                            )
                            nc.vector.tensor_mul(
                                pt[:, 0:8, :, :], pt[:, 0:8, :, :], cmb
                            )
                        else:
                            for qbr in range(8):
                                sl = pt[:, qbr, :, :].rearrange(
                                    "k g qc -> k (g qc)"
                                )
                                nc.gpsimd.tensor_mul(sl, sl, cm2[:, qbr, :])

                for kblk in range(8):
                    ensure_q(kblk)
                    emit_scores(kblk)

                # ---- PV ----
                nw = 8 * (qbw1 - qbw0)
                ob = None
                for g in range(H):
                    pool = ps_qk if g % 2 == 0 else ps_mix
                    tg = "pq" if g % 2 == 0 else "pm"
                    po = pool.tile([128, 512], f32, tag=tg, name="po")
                    for kblk in range(8):
                        qb0 = qb0_of(kblk)
                        nc.tensor.matmul(
                            po[0:65, 8 * (qb0 - qbw0) : 8 * (qbw1 - qbw0)],
                            vt_sb[:, g, kblk, :],
                            probsT[kblk][:, :, g, :],
                            start=(kblk == 0),
                            stop=(kblk == 7),
                        )
                    if g % 4 == 0:
                        ob = ob_pool.tile([128, 4, 512], f32, tag="ob", name="ob")
                    hw_ = nw // 2
                    nc.vector.tensor_copy(ob[0:65, g % 4, 0:hw_], po[0:65, 0:hw_])
                    nc.scalar.copy(ob[0:65, g % 4, hw_:nw], po[0:65, hw_:nw])
                    if g == 13:
                        nc.sync.dma_start(
                            out=out_c[12:14, :, 8 * qbw0 : 8 * qbw1].rearrange(
                                "gg d q -> d gg q"
                            ),
                            in_=ob[0:65, 0:2, 0:nw],
                        )
                    elif g == 15:
                        nc.sync.dma_start(
                            out=out_c[14:16, :, 8 * qbw0 : 8 * qbw1].rearrange(
                                "gg d q -> d gg q"
                            ),
                            in_=ob[0:65, 2:4, 0:nw],
                        )
                    elif g % 4 == 3:
                        nc.sync.dma_start(
                            out=out_c[g - 3 : g + 1, :, 8 * qbw0 : 8 * qbw1].rearrange(
                                "gg d q -> d gg q"
                            ),
                            in_=ob[0:65, :, 0:nw],
                        )

            for widx, (qbw0, qbw1) in enumerate(windows):
                run_window(widx, qbw0, qbw1)

    nc.compile()
    return nc


def _get_nc(causal: int):
    key = int(causal)
    if key not in _compiled:
        _compiled[key] = _build_nc(key)
    return _compiled[key]


def kernel(xq, xk, xv, W, causal):
    import ml_dtypes
    from concourse.bass_utils import run_bass_kernel_spmd

    bf16 = ml_dtypes.bfloat16
    causal = int(np.asarray(causal))
    nc = _get_nc(causal)

    W = np.asarray(W, dtype=np.float32)
    # block-diagonal mixing weight: wblk[8h+qc, 8g+qc] = W[g,h] / 8
    wblk = np.zeros((128, 128), dtype=np.float32)
    for qc in range(8):
        wblk[qc::8, qc::8] = W.T / 8.0
    wblk = wblk.astype(bf16)
    ident = np.eye(128, dtype=np.float32).astype(bf16)

    xq = np.asarray(xq, dtype=np.float32)
    xk = np.asarray(xk, dtype=np.float32)
    xv = np.asarray(xv, dtype=np.float32)

    in_maps = []
    for cid in range(8):
        b, par = divmod(cid, 2)
        # qt[hl*64+d, p, ql] = xq[b, 2p+hl, 2*ql+par, d]
        qt = np.ascontiguousarray(
            xq[b, :, par::2, :]
            .reshape(NP, 2, QC, 64)
            .transpose(1, 3, 0, 2)
            .reshape(128, NP, QC),
            dtype=bf16,
        )
        # kt[hl*64+d, kblk, p, j] = xk[b, 2p+hl, 128*kblk+j, d]
        kt = np.ascontiguousarray(
            xk[b]
            .reshape(NP, 2, 8, 128, 64)
            .transpose(1, 4, 2, 0, 3)
            .reshape(128, 8, NP, 128),
            dtype=bf16,
        )
        # vt[kp, g, kblk, 0:64] = xv[b, g, 128*kblk+kp, :]; vt[..., 64] = 1
        v = xv[b].reshape(H, 8, 128, D).transpose(2, 0, 1, 3)
        vt = np.concatenate(
            [v, np.ones((128, H, 8, 1), dtype=np.float32)], axis=3
        ).astype(bf16)
        # diag mask (g-replicated): cm[kp, qbr, g*8+qc] = 1 if
        # kp <= 16*qbr + 2*qc + par
        kp = np.arange(128)[:, None, None]
        qbr = np.arange(8)[None, :, None]
        qcc = np.arange(8)[None, None, :]
        cm8 = np.where(kp <= 16 * qbr + 2 * qcc + par, 1.0, 0.0).astype(np.float32)
        cm = np.ascontiguousarray(
            np.tile(cm8[:, :, None, :], (1, 1, H, 1)).reshape(128, 8, 128)
        ).astype(bf16)
        in_maps.append(
            {
                "qt_c": qt,
                "kt_c": kt,
                "vt_c": vt,
                "wblk": wblk,
                "cmask": cm,
                "ident": ident,
            }
        )

    global LAST_EXEC_NS, LAST_PROFILE
    res = run_bass_kernel_spmd(nc, in_maps, list(range(8)), trace=TRACE)
    if res.exec_time_ns is not None:
        LAST_EXEC_NS = res.exec_time_ns
        LAST_PROFILE = res.profile_json
    out = np.empty((B, S, H, D), dtype=np.float32)
    for cid in range(8):
        b, par = divmod(cid, 2)
        oc = res.results[cid]["out_c"]  # [H, 65, QC] f32
        o = oc[:, 0:64, :] / oc[:, 64:65, :]
        out[b, par::2, :, :] = o.transpose(2, 0, 1)
    return out


# revision 23
# speedup vs baseline: 1.1274x; 1.0464x over previous
"""HeadConvAttention Trainium2 Bass kernel (transposed-orientation design).

Reference computation (per batch b):
    scores[h,q,k] = <xq[h,q,:], xk[h,k,:]> / sqrt(D)
    mixed[g,q,k]  = sum_h W[g,h] * scores[h,q,k]
    probs         = softmax(mixed + causal_mask, axis=k)
    out[q,g,d]    = sum_k probs[g,q,k] * xv[g,k,d]

Sharding: 8 cores = batch(4) x q-parity(2). Each core handles q rows
{parity, parity+2, ...} (512 rows) of one batch element — strided q keeps
the causal workload balanced across parities.

Everything is kept in the [k, q] ("transposed") orientation so probs come
out of the mix+exp already in the layout PV consumes — no post-softmax
transposes and no PSUM->SBUF probs copies:

  - host pre-transposes inputs: qt [pair, (2h x d), qlocal] bf16,
    kt [pair, (2h x d), k] bf16, vt [kp, g, kblk, d+ones] bf16; W enters as
    the 128x128 block-diag wblk (x 1/8 scale), causal diag mask cm2 and the
    128x128 identity also come from the host.
  - QK^T per (kblk, head pair): lhsT = kt slice [64, 128k], rhs = qt
    [64, q>=64*kblk] -> scoresT [k=128, q] f32 in PSUM, two row-tiled
    matmuls per pair (tile_position (0,0)/(64,0)).
  - scatter (DVE): PSUM -> sc [k, qb, h, qc] bf16 (qb = q//8, qc = q%8).
  - fold (PE transpose per 8-q block): [k, (h qc)] -> [(h qc), k] bf16,
    batched 8 per PSUM bank, one DVE copy out.
  - mix matmul: lhsT = fold slice, rhs = block-diag W -> pm [k, (g qc)]
    f32; exp on ScalarE -> probsT[kblk] [k, g, qb, qc] bf16 (PV-ready,
    (qb, qc) contiguous per g so the PV rhs is a single free dim).
  - causal diagonal masked multiplicatively (mask indep. of kblk:
    kp <= 16*qbr + 2*qc + par); kblk<6 on otherwise-idle GpSimd, the last
    two kblks on DVE so PV is not gated on the slow gpsimd chain.
  - PV per g: lhsT = vt[:, g, kblk, :] (V with a ones column, stationary),
    rhs = probsT[kblk][:, g] streamed (N up to 512), PSUM-accumulated over
    kblk into po [65, q] — row 64 is the softmax denominator. po banks
    alternate between the ps_qk and ps_mix pools (both idle by then) so
    the drain ring never stalls the PE.
  - po -> SBUF -> DMA out [g, 65, q]; host does out = po[:64]/po[64] and
    the final [B, S, H, D] gather.

Emission is software-pipelined (engine queues run in program order):
QKT(kblk+1) is emitted before folds(kblk) so the PE queue has work while
DVE drains the QKT PSUM banks; fold groups and mix groups interleave.
Matmul-weights APs must have a single free dim, and PSUM tiles are
bank-sized — these constraints fix the sc/probsT layouts above.
"""

import numpy as np

B, H, S, D = 4, 16, 1024, 64
QC = S // 2          # q rows per core
NP = 8               # head pairs

_compiled = {}
TRACE = False          # set True to capture an NTFF profile on the next call
LAST_EXEC_NS = None
LAST_PROFILE = None


def _build_nc(causal: int):
    import contextlib

    import concourse.bacc as bacc
    import concourse.bass as bass
    import concourse.mybir as mybir
    import concourse.tile as tile

    dt = mybir.dt
    f32, bf16 = dt.float32, dt.bfloat16
    AF = mybir.ActivationFunctionType

    nc = bacc.Bacc("TRN2", target_bir_lowering=False, debug=False, num_devices=8)

    qt_c = nc.dram_tensor("qt_c", [128, NP, QC], bf16, kind="ExternalInput")
    kt_c = nc.dram_tensor("kt_c", [128, 8, NP, 128], bf16, kind="ExternalInput")
    vt_c = nc.dram_tensor("vt_c", [128, H, 8, 65], bf16, kind="ExternalInput")
    wblk = nc.dram_tensor("wblk", [128, 128], bf16, kind="ExternalInput")
    cmask = nc.dram_tensor("cmask", [128, 8, 128], bf16, kind="ExternalInput")
    ident = nc.dram_tensor("ident", [128, 128], bf16, kind="ExternalInput")
    out_c = nc.dram_tensor("out_c", [H, 65, QC], f32, kind="ExternalOutput")

    # q windows (qb units of 8 local q rows). causal=1 fits one window in
    # SBUF because probsT shrinks with kblk; causal=0 needs two passes.
    if causal:
        windows = [(0, QC // 8)]
    else:
        windows = [(0, QC // 16), (QC // 16, QC // 8)]

    with tile.TileContext(nc) as tc:
        with contextlib.ExitStack() as ctx:
            const = ctx.enter_context(tc.tile_pool(name="const", bufs=1))
            persist = ctx.enter_context(tc.tile_pool(name="persist", bufs=1))
            sc_pool = ctx.enter_context(tc.tile_pool(name="scores", bufs=3))
            fold_pool = ctx.enter_context(tc.tile_pool(name="fold", bufs=4))
            ob_pool = ctx.enter_context(tc.tile_pool(name="outp", bufs=3))
            # PSUM budget: 8 banks. qk/po 3 + ft 2 + mix 3 = 8.
            ps_qk = ctx.enter_context(tc.tile_pool(name="ps_qk", bufs=3, space="PSUM"))
            ps_ft = ctx.enter_context(tc.tile_pool(name="ps_ft", bufs=2, space="PSUM"))
            ps_mix = ctx.enter_context(tc.tile_pool(name="ps_mix", bufs=3, space="PSUM"))

            # ---- input loads (batched; kblk-0 K slice + first qt half first) ----
            qt_a = persist.tile([128, 4, QC], bf16, tag="qt_a")
            qt_b = persist.tile([128, 4, QC], bf16, tag="qt_b")
            kt0_sb = persist.tile([128, NP, 128], bf16, tag="kt0")
            ktr_sb = persist.tile([128, 7, NP, 128], bf16, tag="ktr")
            nc.sync.dma_start(out=kt0_sb, in_=kt_c[:, 0, :, :])
            nc.sync.dma_start(out=qt_a, in_=qt_c[:, 0:4, :])
            nc.sync.dma_start(out=qt_b, in_=qt_c[:, 4:8, :])

            # ---- constants ----
            id_bf = const.tile([128, 128], bf16, tag="id_bf")
            nc.sync.dma_start(out=id_bf, in_=ident[:, :])
            wblk_bf = const.tile([128, 128], bf16, tag="wblk_bf")
            nc.sync.dma_start(out=wblk_bf, in_=wblk[:, :])
            cm2 = const.tile([128, 8, 128], bf16, tag="cm2")

            def qt_slice(hl, p, c0, c1):
                t = qt_a if p < 4 else qt_b
                return t[64 * hl : 64 * (hl + 1), p % 4, c0:c1]
            nc.sync.dma_start(
                out=ktr_sb.rearrange("k a p j -> k (a p j)"),
                in_=kt_c[:, 1:8, :, :].rearrange("k a p j -> k (a p j)"),
            )
            nc.sync.dma_start(out=cm2, in_=cmask[:, :, :])
            vt_sb = persist.tile([128, H, 8, 65], bf16, tag="vt")
            nc.sync.dma_start(out=vt_sb, in_=vt_c[:, :, :, :])

            def kt_slice(p, kblk):
                if kblk == 0:
                    return kt0_sb[:, p, :]
                return ktr_sb[:, kblk - 1, p, :]

            def run_window(widx, qbw0, qbw1):
                # probsT[kblk]: [k, g, qbr, qc] bf16, qbr local to window
                probsT = {}
                sc_tiles = {}
                q_emitted = set()

                def qb0_of(kblk):
                    return max(qbw0, 8 * kblk) if causal else qbw0

                def emit_qkt_pair(kblk, p):
                    qb0 = qb0_of(kblk)
                    nqb = qbw1 - qb0
                    nq = 8 * nqb
                    if kblk not in sc_tiles:
                        sc_tiles[kblk] = sc_pool.tile(
                            [128, qbw1 - qbw0, H, 8], bf16, tag="sc", name="sc"
                        )
                    sc = sc_tiles[kblk]
                    pq0 = ps_qk.tile([128, 512], f32, tag="pq", name="pq0")[:, 0:nq]
                    pq1 = ps_qk.tile([128, 512], f32, tag="pq", name="pq1")[:, 0:nq]
                    nc.tensor.matmul(
                        pq0,
                        kt_slice(p, kblk)[0:64, :],
                        qt_slice(0, p, 8 * qb0, 8 * qbw1),
                        start=True,
                        stop=True,
                        tile_position=(0, 0),
                    )
                    nc.tensor.matmul(
                        pq1,
                        kt_slice(p, kblk)[64:128, :],
                        qt_slice(1, p, 8 * qb0, 8 * qbw1),
                        start=True,
                        stop=True,
                        tile_position=(64, 0),
                    )
                    src0 = pq0.rearrange("k (qb qc) -> k qb qc", qc=8)
                    src1 = pq1.rearrange("k (qb qc) -> k qb qc", qc=8)
                    nc.vector.tensor_copy(sc[:, 0:nqb, 2 * p, :], src0)
                    nc.vector.tensor_copy(sc[:, 0:nqb, 2 * p + 1, :], src1)

                def ensure_q(kblk):
                    if 0 <= kblk < 8 and kblk not in q_emitted:
                        for p in range(NP):
                            emit_qkt_pair(kblk, p)
                        q_emitted.add(kblk)

                def emit_fold_grp(kblk, grp):
                    sc = sc_tiles[kblk]
                    ft = ps_ft.tile([128, 8, 128], bf16, tag="ft", name="ft")
                    for i in range(8):
                        nc.tensor.transpose(
                            ft[:, i, :],
                            sc[:, grp * 8 + i, :, :].rearrange("k h qc -> k (h qc)"),
                            id_bf,
                        )
                    fold_sb = fold_pool.tile([128, 8, 128], bf16, tag="fold", name="fold_sb")
                    # middle kblks are ScalarE-bound (exp) -> lean on DVE there
                    s = 6 if 3 <= kblk <= 6 else 4
                    nc.vector.tensor_copy(fold_sb[:, 0:s, :], ft[:, 0:s, :])
                    nc.scalar.copy(fold_sb[:, s:8, :], ft[:, s:8, :])
                    return fold_sb

                def emit_mix_grp(kblk, grp, fold_sb):
                    pt = probsT[kblk]
                    for half in range(2):
                        pm = ps_mix.tile([128, 512], f32, tag="pm", name="pm")
                        for i in range(4):
                            nc.tensor.matmul(
                                pm[:, 128 * i : 128 * (i + 1)],
                                fold_sb[:, half * 4 + i, :],
                                wblk_bf,
                                start=True,
                                stop=True,
                            )
                        qbr0 = grp * 8 + half * 4
                        nc.scalar.activation(
                            pt[:, qbr0 : qbr0 + 4, :, :].rearrange(
                                "k qb g qc -> k qb (g qc)"
                            ),
                            pm.rearrange("k (qb gq) -> k qb gq", gq=128),
                            AF.Exp,
                        )

                def emit_scores(kblk):
                    qb0 = qb0_of(kblk)
                    nqb = qbw1 - qb0
                    probsT[kblk] = persist.tile(
                        [128, nqb, H, 8], bf16, tag=f"pt_{widx}_{kblk}",
                        name=f"pt_{widx}_{kblk}",
                    )
                    ngrp = nqb // 8
                    # interleave next kblk's QKT pairs between fold groups so
                    # the PE queue never blocks on the pq ring at boundaries
                    nxt = kblk + 1
                    do_next = nxt < 8 and nxt not in q_emitted
                    pend = None
                    for grp in range(ngrp):
                        fold_sb = emit_fold_grp(kblk, grp)
                        if do_next:
                            lo = NP * grp // ngrp
                            hi = NP * (grp + 1) // ngrp
                            for p in range(lo, hi):
                                emit_qkt_pair(nxt, p)
                        if pend is not None:
                            emit_mix_grp(kblk, pend[0], pend[1])
                        pend = (grp, fold_sb)
                    if do_next:
                        q_emitted.add(nxt)
                    if pend is not None:
                        emit_mix_grp(kblk, pend[0], pend[1])
                    sc_tiles.pop(kblk)
                    if causal and 8 * kblk >= qbw0:
                        # diagonal 8 qb straddle the causal boundary; the two
                        # last kblks go to DVE so PV is not gated on the slow
                        # gpsimd mask chain
                        pt = probsT[kblk]
                        if kblk >= 6:
                            cmb = cm2[:, :, :].rearrange(
                                "k qb (g qc) -> k qb g qc", qc=8
                            )
                            nc.vector.tensor_mul(
                                pt[:, 0:8, :, :], pt[:, 0:8, :, :], cmb
                            )
                        else:
                            for qbr in range(8):
                                sl = pt[:, qbr, :, :].rearrange(
                                    "k g qc -> k (g qc)"
                                )
                                nc.gpsimd.tensor_mul(sl, sl, cm2[:, qbr, :])

                for kblk in range(8):
                    ensure_q(kblk)
                    emit_scores(kblk)

                # ---- PV ----
                nw = 8 * (qbw1 - qbw0)
                ob = None
                for g in range(H):
                    pool = ps_qk if g % 2 == 0 else ps_mix
                    tg = "pq" if g % 2 == 0 else "pm"
                    po = pool.tile([128, 512], f32, tag=tg, name="po")
                    for kblk in range(8):
                        qb0 = qb0_of(kblk)
                        nc.tensor.matmul(
                            po[0:65, 8 * (qb0 - qbw0) : 8 * (qbw1 - qbw0)],
                            vt_sb[:, g, kblk, :],
                            probsT[kblk][:, :, g, :],
                            start=(kblk == 0),
                            stop=(kblk == 7),
                        )
                    if g % 4 == 0:
                        ob = ob_pool.tile([128, 4, 512], f32, tag="ob", name="ob")
                    hw_ = nw // 2
                    nc.vector.tensor_copy(ob[0:65, g % 4, 0:hw_], po[0:65, 0:hw_])
                    nc.scalar.copy(ob[0:65, g % 4, hw_:nw], po[0:65, hw_:nw])
                    if g == 13:
                        nc.sync.dma_start(
                            out=out_c[12:14, :, 8 * qbw0 : 8 * qbw1].rearrange(
                                "gg d q -> d gg q"
                            ),
                            in_=ob[0:65, 0:2, 0:nw],
                        )
                    elif g == 15:
                        nc.sync.dma_start(
                            out=out_c[14:16, :, 8 * qbw0 : 8 * qbw1].rearrange(
                                "gg d q -> d gg q"
                            ),
                            in_=ob[0:65, 2:4, 0:nw],
                        )
                    elif g % 4 == 3:
                        nc.sync.dma_start(
                            out=out_c[g - 3 : g + 1, :, 8 * qbw0 : 8 * qbw1].rearrange(
                                "gg d q -> d gg q"
                            ),
                            in_=ob[0:65, :, 0:nw],
                        )

            for widx, (qbw0, qbw1) in enumerate(windows):
                run_window(widx, qbw0, qbw1)

    nc.compile()
    return nc


def _get_nc(causal: int):
    key = int(causal)
    if key not in _compiled:
        _compiled[key] = _build_nc(key)
    return _compiled[key]


def kernel(xq, xk, xv, W, causal):
    import ml_dtypes
    from concourse.bass_utils import run_bass_kernel_spmd

    bf16 = ml_dtypes.bfloat16
    causal = int(np.asarray(causal))
    nc = _get_nc(causal)

    W = np.asarray(W, dtype=np.float32)
    # block-diagonal mixing weight: wblk[8h+qc, 8g+qc] = W[g,h] / 8
    wblk = np.zeros((128, 128), dtype=np.float32)
    for qc in range(8):
        wblk[qc::8, qc::8] = W.T / 8.0
    wblk = wblk.astype(bf16)
    ident = np.eye(128, dtype=np.float32).astype(bf16)

    xq = np.asarray(xq, dtype=np.float32)
    xk = np.asarray(xk, dtype=np.float32)
    xv = np.asarray(xv, dtype=np.float32)

    in_maps = []
    for cid in range(8):
        b, par = divmod(cid, 2)
        # qt[hl*64+d, p, ql] = xq[b, 2p+hl, 2*ql+par, d]
        qt = np.ascontiguousarray(
            xq[b, :, par::2, :]
            .reshape(NP, 2, QC, 64)
            .transpose(1, 3, 0, 2)
            .reshape(128, NP, QC),
            dtype=bf16,
        )
        # kt[hl*64+d, kblk, p, j] = xk[b, 2p+hl, 128*kblk+j, d]
        kt = np.ascontiguousarray(
            xk[b]
            .reshape(NP, 2, 8, 128, 64)
            .transpose(1, 4, 2, 0, 3)
            .reshape(128, 8, NP, 128),
            dtype=bf16,
        )
        # vt[kp, g, kblk, 0:64] = xv[b, g, 128*kblk+kp, :]; vt[..., 64] = 1
        v = xv[b].reshape(H, 8, 128, D).transpose(2, 0, 1, 3)
        vt = np.concatenate(
            [v, np.ones((128, H, 8, 1), dtype=np.float32)], axis=3
        ).astype(bf16)
        # diag mask (g-replicated): cm[kp, qbr, g*8+qc] = 1 if
        # kp <= 16*qbr + 2*qc + par
        kp = np.arange(128)[:, None, None]
        qbr = np.arange(8)[None, :, None]
        qcc = np.arange(8)[None, None, :]
        cm8 = np.where(kp <= 16 * qbr + 2 * qcc + par, 1.0, 0.0).astype(np.float32)
        cm = np.ascontiguousarray(
            np.tile(cm8[:, :, None, :], (1, 1, H, 1)).reshape(128, 8, 128)
        ).astype(bf16)
        in_maps.append(
            {
                "qt_c": qt,
                "kt_c": kt,
                "vt_c": vt,
                "wblk": wblk,
                "cmask": cm,
                "ident": ident,
            }
        )

    global LAST_EXEC_NS, LAST_PROFILE
    res = run_bass_kernel_spmd(nc, in_maps, list(range(8)), trace=TRACE)
    if res.exec_time_ns is not None:
        LAST_EXEC_NS = res.exec_time_ns
        LAST_PROFILE = res.profile_json
    out = np.empty((B, S, H, D), dtype=np.float32)
    for cid in range(8):
        b, par = divmod(cid, 2)
        oc = res.results[cid]["out_c"]  # [H, 65, QC] f32
        o = oc[:, 0:64, :] / oc[:, 64:65, :]
        out[b, par::2, :, :] = o.transpose(2, 0, 1)
    return out
